# revision 8
# baseline (speedup 1.0000x reference)
"""Trainium2 Bass kernel: 16-head MHA (S=4096, D=1024) sharded 2 heads/core over 8 cores.

v2 redesign (HW-microbenchmarked):
  - PE row-tile alternation: dk=64 matmuls with tile_position rows alternating
    0/64 run on the two independent 64x128 sub-arrays CONCURRENTLY (measured
    116ns vs 437ns per 512-col matmul).  QK alternates via the packed-head
    layout plus partition-shifted duplicates (kdup/qd, SBUF->SBUF DMA); PV
    alternates by splitting each t-block's contraction into two 64-row halves
    accumulating into the same PSUM tile.
  - q/k projections and output projection pack both heads into one K=128
    matmul stream (halves matmul count vs per-head K=64).
  - phases: A) DMA-staged k/v projections for all S; B) per s-block: q
    projection + attention (GROUPS-pipelined QK->exp->PV) + output projection.
  - exp on ScalarE psum->sbuf f16 (measured ~2.54us per [128,2048] tile; ACT
    is the roofline at ~325us/core).
  - out partials f16, summed on host in f32; v-bias applied on host.
"""

import sys

for _p in ("/opt/trn_rl_repo",):
    if _p not in sys.path:
        sys.path.insert(0, _p)

import numpy as np
import ml_dtypes

import concourse.bass as bass  # noqa: F401
import concourse.mybir as mybir
import concourse.tile as tile
from concourse import bacc
from concourse.bass_utils import run_bass_kernel_spmd

P = 128
S = 4096
D = 1024
H = 16
DK = 64
HL = 2            # heads per core
NC = 8            # cores
SB = 512          # s-block width
NSB = S // SB     # 8
TB = 128          # t-block (scores partition dim)
NTB = S // TB     # 32
DO = D // P       # 8 d-chunks
GROUPS = [2, 4, 2, 4, 2, 4, 2, 4, 2, 4, 2]
assert sum(GROUPS) == NTB
PV_LAG = 4
PV_ALT = False     # split PV t-contraction into alternating 64-row sub-arrays

F32 = mybir.dt.float32
F16 = mybir.dt.float16
BF16 = mybir.dt.bfloat16
BF16_NP = ml_dtypes.bfloat16
F16_NP = np.float16


def build_nc(reps: int = 1, loop_n: int = 0, phase1: bool = True, phase2: bool = True):
    from contextlib import ExitStack

    nc = bacc.Bacc("TRN2", target_bir_lowering=False, debug=False, num_devices=NC)
    qt = nc.dram_tensor("qt", [D, S], BF16, kind="ExternalInput").ap()
    kt = nc.dram_tensor("kt", [D, S], BF16, kind="ExternalInput").ap()
    vt = nc.dram_tensor("vt", [D, S], BF16, kind="ExternalInput").ap()
    wqk = nc.dram_tensor("wqk", [P, DO, 2, P], BF16, kind="ExternalInput").ap()
    wv = nc.dram_tensor("wv", [P, DO, P], BF16, kind="ExternalInput").ap()
    wot = nc.dram_tensor("wot", [P, D], BF16, kind="ExternalInput").ap()
    bqk = nc.dram_tensor("bqk", [P, 2], F32, kind="ExternalInput").ap()
    wqkS = nc.dram_tensor("wqkS", [P, DO, 2, P], BF16, kind="ExternalInput").ap()
    bqkS = nc.dram_tensor("bqkS", [P, 2], F32, kind="ExternalInput").ap()
    out = nc.dram_tensor("out", [S, D], F16, kind="ExternalOutput").ap()
    _KVER = 21
    vw = 16 + 64 * _KVER + 4 * loop_n + reps + (0 if phase1 else 1) + (0 if phase2 else 2)
    ver = nc.dram_tensor("ver", [1, vw], F32, kind="ExternalOutput").ap()

    qt_r = qt.rearrange("(o p) s -> p o s", p=P)
    kt_r = kt.rearrange("(o p) s -> p o s", p=P)
    vt_r = vt.rearrange("(o p) s -> p o s", p=P)
    out_r = out.rearrange("(so p) m -> p so m", p=P)  # [128, 32, 1024]

    with tile.TileContext(nc) as tc, ExitStack() as ctx:
        const = ctx.enter_context(tc.tile_pool(name="const", bufs=1))
        pers = ctx.enter_context(tc.tile_pool(name="pers", bufs=1))
        pin = ctx.enter_context(tc.tile_pool(name="pin", bufs=4))
        pqd = ctx.enter_context(tc.tile_pool(name="pqd", bufs=2))
        pxs = ctx.enter_context(tc.tile_pool(name="pxs", bufs=2))
        pP = ctx.enter_context(tc.tile_pool(name="pP", bufs=5))
        pout = ctx.enter_context(tc.tile_pool(name="pout", bufs=3))
        prr = ctx.enter_context(tc.tile_pool(name="prr", bufs=2))
        psA = ctx.enter_context(tc.tile_pool(name="psA", bufs=1, space="PSUM"))
        psB = ctx.enter_context(tc.tile_pool(name="psB", bufs=1, space="PSUM"))
        psO = ctx.enter_context(tc.tile_pool(name="psO", bufs=1, space="PSUM"))
        psV = ctx.enter_context(tc.tile_pool(name="psV", bufs=1, space="PSUM"))

        wqk_sb = const.tile([P, DO, 2, P], BF16)
        nc.sync.dma_start(wqk_sb[:], wqk)
        wv_sb = const.tile([P, DO, P], BF16)
        nc.sync.dma_start(wv_sb[:], wv)
        wot_sb = const.tile([P, D], BF16)
        nc.sync.dma_start(wot_sb[:], wot)
        bqk_sb = const.tile([P, 2], F32)
        nc.sync.dma_start(bqk_sb[:], bqk)
        wqkS_sb = const.tile([P, DO, 2, P], BF16)
        nc.sync.dma_start(wqkS_sb[:], wqkS)
        bqkS_sb = const.tile([P, 2], F32)
        nc.sync.dma_start(bqkS_sb[:], bqkS)
        ones_sb = const.tile([1, DK], F32)
        nc.vector.memset(ones_sb[:], 1.0)
        ver_sb = const.tile([1, vw], F32)
        nc.vector.memset(ver_sb[:], float(vw))
        nc.sync.dma_start(ver, ver_sb[:])

        def body():
            qT = pers.tile([P, S], BF16, tag="qT", name="qT")
            kT = pers.tile([P, S], BF16, tag="kT", name="kT")
            kdup = pers.tile([P, S], BF16, tag="kdup", name="kdup")
            vx0 = pers.tile([P, NTB, DK + 1], F16, tag="vx0", name="vx0")
            vx1 = pers.tile([P, NTB, DK + 1], F16, tag="vx1", name="vx1")
            xT = pers.tile([P, S], BF16, tag="xT", name="xT")
            nc.vector.memset(vx0[:, :, DK], 1.0)
            nc.vector.memset(vx1[:, :, DK], 1.0)
            if not phase1:
                nc.vector.memset(qT[:], 0.01)
                nc.vector.memset(kT[:], 0.01)
                nc.vector.memset(kdup[:], 0.01)
                nc.vector.memset(vx0[:, :, 0:DK], 0.01)
                nc.vector.memset(vx1[:, :, 0:DK], 0.01)

            # ---------------- phase A: k + v projections ----------------
            for sb in range(NSB if phase1 else 0):
                s0 = sb * SB
                ks = pin.tile([P, DO, SB], BF16, tag="ks", name="ks")
                nc.sync.dma_start(ks[:], kt_r[:, :, s0 : s0 + SB])
                vs = pin.tile([P, DO, SB], BF16, tag="vs", name="vs")
                nc.sync.dma_start(vs[:], vt_r[:, :, s0 : s0 + SB])
                pk = psB.tile([P, 2 * SB], F32, tag="sB", name="pk")
                for o in range(DO):
                    nc.tensor.matmul(
                        pk[:, 0:SB], wqk_sb[:, o, 1, :], ks[:, o],
                        start=(o == 0), stop=(o == DO - 1),
                    )
                nc.vector.tensor_scalar(
                    kT[:, s0 : s0 + SB], pk[:, 0:SB],
                    bqk_sb[:, 1:2], None, mybir.AluOpType.add,
                )
                # head-swapped duplicate for QK row-tile alternation (no DMA)
                pk2 = psA.tile([P, 4 * SB], F32, tag="sA", name="pk2")
                for o in range(DO):
                    nc.tensor.matmul(
                        pk2[:, 0:SB], wqkS_sb[:, o, 1, :], ks[:, o],
                        start=(o == 0), stop=(o == DO - 1),
                    )
                nc.vector.tensor_scalar(
                    kdup[:, s0 : s0 + SB], pk2[:, 0:SB],
                    bqkS_sb[:, 1:2], None, mybir.AluOpType.add,
                )

                for tb in range(SB // TB):
                    if tb % 2 == 0:
                        pv = psV.tile([P, SB], F32, tag="pv", name="pvv")
                    else:
                        pv = psO.tile([P, SB], F32, tag="oT", name="pvo")
                    pvv = pv[:, 0:P]
                    tbg = sb * (SB // TB) + tb
                    for o in range(DO):
                        nc.tensor.matmul(
                            pvv, vs[:, o, tb * TB : (tb + 1) * TB], wv_sb[:, o],
                            start=(o == 0), stop=(o == DO - 1),
                        )
                    nc.vector.tensor_copy(vx0[:, tbg, 0:DK], pvv[:, 0:DK])
                    nc.vector.tensor_copy(vx1[:, tbg, 0:DK], pvv[:, DK:P])

            def do_outproj(sbp):
                for si in range(SB // P):
                    so = sbp * (SB // P) + si
                    for mb in range(2):
                        m0 = mb * SB
                        op = psV.tile([P, SB], F32, tag="pv", name="op")
                        nc.tensor.matmul(
                            op, xT[:, so * P : (so + 1) * P],
                            wot_sb[:, m0 : m0 + SB],
                            start=True, stop=True,
                        )
                        ob = pout.tile([P, SB], F16, tag="ob", name="ob")
                        nc.vector.tensor_copy(ob[:], op)
                        nc.sync.dma_start(out_r[:, so, m0 : m0 + SB], ob[:])

            # -------- phase B: q proj + attention + output projection --------
            for sb in range(NSB if phase2 else 0):
                s0 = sb * SB
                qs = pin.tile([P, DO, SB], BF16, tag="qs", name="qs")
                nc.sync.dma_start(qs[:], qt_r[:, :, s0 : s0 + SB])
                pq = psA.tile([P, 4 * SB], F32, tag="sA", name="pq")
                for o in range(DO):
                    nc.tensor.matmul(
                        pq[:, 0:SB], wqk_sb[:, o, 0, :], qs[:, o],
                        start=(o == 0), stop=(o == DO - 1),
                    )
                for o in range(DO):
                    nc.tensor.matmul(
                        pq[:, SB : 2 * SB], wqkS_sb[:, o, 0, :], qs[:, o],
                        start=(o == 0), stop=(o == DO - 1),
                    )
                nc.vector.tensor_scalar(
                    qT[:, s0 : s0 + SB], pq[:, 0:SB],
                    bqk_sb[:, 0:1], 0.125,
                    mybir.AluOpType.add, mybir.AluOpType.mult,
                )
                qd = pqd.tile([P, SB], BF16, tag="qd", name="qd")
                nc.vector.tensor_scalar(
                    qd[:], pq[:, SB : 2 * SB],
                    bqkS_sb[:, 0:1], 0.125,
                    mybir.AluOpType.add, mybir.AluOpType.mult,
                )

                for h in (1, 0):  # h1 first: its xT DMA-shift hides under h0
                    vx = vx0 if h == 0 else vx1
                    oT = psO.tile([DK + 1, SB], F32, tag="oT", name="oT")
                    starts = []
                    t = 0
                    for g in GROUPS:
                        starts.append(t)
                        t += g
                    pts = [None] * len(GROUPS)
                    NG = len(GROUPS)

                    def do_pv(gi):
                        g, gs = GROUPS[gi], starts[gi]
                        for i in range(g):
                            tb = gs + i
                            if PV_ALT:
                                for half in range(2):
                                    r0 = half * DK
                                    nc.tensor.matmul(
                                        oT, vx[r0 : r0 + DK, tb, :],
                                        pts[gi][r0 : r0 + DK, i * SB : (i + 1) * SB],
                                        start=(tb == 0 and half == 0),
                                        stop=(tb == NTB - 1 and half == 1),
                                    )
                            else:
                                nc.tensor.matmul(
                                    oT, vx[:, tb, :], pts[gi][:, i * SB : (i + 1) * SB],
                                    start=(tb == 0), stop=(tb == NTB - 1),
                                )

                    for gi, g in enumerate(GROUPS):
                        gs = starts[gi]
                        pool, width = (psB, 2 * SB) if g == 2 else (psA, 4 * SB)
                        sc = pool.tile([P, width], F32, tag=("sB" if g == 2 else "sA"), name="sc")
                        for i in range(g):
                            tb = gs + i
                            # row-tile position: h0 even tb -> 0, odd -> 64;
                            # h1 even tb -> 0 (kdup), odd -> 64 (kT)
                            pos = (tb % 2) * DK
                            native = pos == h * DK
                            k_src = kT if native else kdup
                            if native:
                                q_op = qT[pos : pos + DK, s0 : s0 + SB]
                            else:
                                q_op = qd[pos : pos + DK, :]
                            nc.tensor.matmul(
                                sc[:, i * SB : (i + 1) * SB],
                                k_src[pos : pos + DK, tb * TB : (tb + 1) * TB],
                                q_op,
                                start=True, stop=True,
                            )
                        pt = pP.tile([P, 4 * SB], F16, tag="P", name="pt")
                        nc.scalar.activation(
                            pt[:, : g * SB], sc[:, : g * SB],
                            mybir.ActivationFunctionType.Exp,
                        )
                        pts[gi] = pt
                        if gi >= PV_LAG:
                            do_pv(gi - PV_LAG)
                    for gi in range(NG - PV_LAG, NG):
                        do_pv(gi)

                    # r = 1/l ; broadcast over 64 partitions ; xT = oT * r
                    r_t = prr.tile([1, SB], F32, tag="r", name="r_t")
                    r_s = prr.tile([1, SB], F32, tag="rs", name="r_s")
                    l_t = prr.tile([1, SB], F32, tag="lt", name="l_t")
                    nc.vector.tensor_copy(l_t[:], oT[DK : DK + 1, :])
                    nc.vector.reciprocal_approx_accurate(r_t[:], l_t[:], r_s[:])
                    bc = psV.tile([P, SB], F32, tag="pv", name="bc")
                    nc.tensor.matmul(bc[0:DK, :], ones_sb[:], r_t[:], start=True, stop=True)
                    bc_sb = prr.tile([DK, SB], F32, tag="bcs", name="bc_sb")
                    nc.vector.tensor_copy(bc_sb[:], bc[0:DK, :])
                    if h == 0:
                        nc.vector.tensor_tensor(
                            xT[0:DK, s0 : s0 + SB], oT[0:DK, :], bc_sb[:],
                            mybir.AluOpType.mult,
                        )
                    else:
                        xst = pxs.tile([DK, SB], BF16, tag="xst", name="xst")
                        nc.vector.tensor_tensor(
                            xst[:], oT[0:DK, :], bc_sb[:], mybir.AluOpType.mult
                        )
                        nc.sync.dma_start(xT[DK:P, s0 : s0 + SB], xst[:])

                # output projection delayed one s-block: xT(sb-1) has had a
                # full attention pair of slack (covers the h1 xst DMA shift)
                if sb > 0:
                    do_outproj(sb - 1)
            if phase2:
                do_outproj(NSB - 1)

        if loop_n > 0:
            with tc.For_i(0, loop_n, 1):
                body()
        else:
            for _ in range(reps):
                body()

    nc.finalize()
    return nc


def _pack_core_inputs(c, QT, KT, VT, Wq, bq, Wk, bk, Wv, Wo):
    """Per-core input dict (core c owns heads 2c, 2c+1)."""
    h0 = HL * c
    # wqk[p, o, qk, j]: j = (head, dk) packed
    wqk = np.zeros((P, DO, 2, P), dtype=BF16_NP)
    for hh in range(HL):
        wqk[:, :, 0, hh * DK : (hh + 1) * DK] = (
            Wq[h0 + hh].reshape(DO, P, DK).transpose(1, 0, 2).astype(BF16_NP)
        )
        wqk[:, :, 1, hh * DK : (hh + 1) * DK] = (
            Wk[h0 + hh].reshape(DO, P, DK).transpose(1, 0, 2).astype(BF16_NP)
        )
    wv = (
        Wv[h0 : h0 + HL].reshape(HL, DO, P, DK).transpose(2, 1, 0, 3).reshape(P, DO, P)
    ).astype(BF16_NP)
    # wot[j, m]: rows = (head, dk) packed
    wot = np.ascontiguousarray(Wo[:, h0 * DK : (h0 + HL) * DK].T).astype(BF16_NP)
    bqk = np.zeros((P, 2), dtype=np.float32)
    for hh in range(HL):
        bqk[hh * DK : (hh + 1) * DK, 0] = bq[h0 + hh]
        bqk[hh * DK : (hh + 1) * DK, 1] = bk[h0 + hh]
    # head-swapped copies (duplicate rows for PE row-tile alternation)
    wqkS = np.concatenate([wqk[:, :, :, DK:], wqk[:, :, :, :DK]], axis=3)
    bqkS = np.concatenate([bqk[DK:], bqk[:DK]], axis=0)
    return {
        "qt": QT, "kt": KT, "vt": VT,
        "wqk": np.ascontiguousarray(wqk),
        "wv": np.ascontiguousarray(wv),
        "wot": wot,
        "bqk": np.ascontiguousarray(bqk),
        "wqkS": np.ascontiguousarray(wqkS),
        "bqkS": np.ascontiguousarray(bqkS),
    }


def make_in_maps(Q, K, V, Wq, bq, Wk, bk, Wv, bv, Wo, bo):
    QT = np.ascontiguousarray(Q.T).astype(BF16_NP)
    KT = np.ascontiguousarray(K.T).astype(BF16_NP)
    VT = np.ascontiguousarray(V.T).astype(BF16_NP)
    return [
        _pack_core_inputs(c, QT, KT, VT, Wq, bq, Wk, bk, Wv, Wo) for c in range(NC)
    ]


def host_combine(partials, Wq, bv, Wo, bo):
    total = np.zeros((S, D), np.float32)
    for p in partials:
        total += p.astype(np.float32)
    # v-bias passes through softmax exactly as +bv on the concat features
    total += bv.reshape(-1).astype(np.float32) @ Wo.T.astype(np.float32) + bo
    return total


_NC_CACHE = {}


def _get_nc(reps=1):
    if reps not in _NC_CACHE:
        _NC_CACHE[reps] = build_nc(reps)
    return _NC_CACHE[reps]


def kernel(Q, K, V, Wq, bq, Wk, bk, Wv, bv, Wo, bo):
    args = [np.asarray(x) for x in (Q, K, V, Wq, bq, Wk, bk, Wv, bv, Wo, bo)]
    Q, K, V, Wq, bq, Wk, bk, Wv, bv, Wo, bo = args
    nc = _get_nc()
    in_maps = make_in_maps(Q, K, V, Wq, bq, Wk, bk, Wv, bv, Wo, bo)
    res = run_bass_kernel_spmd(nc, in_maps, core_ids=list(range(NC)))
    partials = [res.results[c]["out"] for c in range(NC)]
    return host_combine(partials, Wq, bv, Wo, bo)


# revision 9
# speedup vs baseline: 1.1902x; 1.1902x over previous
"""Trainium2 Bass kernel: 16-head MHA (S=4096, D=1024) sharded 2 heads/core over 8 cores.

v2 redesign (HW-microbenchmarked):
  - PE row-tile alternation: dk=64 matmuls with tile_position rows alternating
    0/64 run on the two independent 64x128 sub-arrays CONCURRENTLY (measured
    116ns vs 437ns per 512-col matmul).  QK alternates via the packed-head
    layout plus partition-shifted duplicates (kdup/qd, SBUF->SBUF DMA); PV
    alternates by splitting each t-block's contraction into two 64-row halves
    accumulating into the same PSUM tile.
  - q/k projections and output projection pack both heads into one K=128
    matmul stream (halves matmul count vs per-head K=64).
  - phases: A) DMA-staged k/v projections for all S; B) per s-block: q
    projection + attention (GROUPS-pipelined QK->exp->PV) + output projection.
  - exp on ScalarE psum->sbuf f16 (measured ~2.54us per [128,2048] tile; ACT
    is the roofline at ~325us/core).
  - out partials f16, summed on host in f32; v-bias applied on host.
"""

import sys

for _p in ("/opt/trn_rl_repo",):
    if _p not in sys.path:
        sys.path.insert(0, _p)

import numpy as np
import ml_dtypes

import concourse.bass as bass  # noqa: F401
import concourse.mybir as mybir
import concourse.tile as tile
from concourse import bacc
from concourse.bass_utils import run_bass_kernel_spmd

P = 128
S = 4096
D = 1024
H = 16
DK = 64
HL = 2            # heads per core
NC = 8            # cores
SB = 512          # s-block width
NSB = S // SB     # 8
TB = 128          # t-block (scores partition dim)
NTB = S // TB     # 32
DO = D // P       # 8 d-chunks
GROUPS = [2, 4, 2, 4, 2, 4, 2, 4, 2, 4, 2]
assert sum(GROUPS) == NTB
PV_LAG = 3
PV_ALT = False     # split PV t-contraction into alternating 64-row sub-arrays

F32 = mybir.dt.float32
F16 = mybir.dt.float16
BF16 = mybir.dt.bfloat16
BF16_NP = ml_dtypes.bfloat16
F16_NP = np.float16


def build_nc(reps: int = 1, loop_n: int = 0, phase1: bool = True, phase2: bool = True):
    from contextlib import ExitStack

    nc = bacc.Bacc("TRN2", target_bir_lowering=False, debug=False, num_devices=NC)
    qt = nc.dram_tensor("qt", [D, S], BF16, kind="ExternalInput").ap()
    kt = nc.dram_tensor("kt", [D, S], BF16, kind="ExternalInput").ap()
    vt = nc.dram_tensor("vt", [D, S], BF16, kind="ExternalInput").ap()
    wqk = nc.dram_tensor("wqk", [P, DO, 2, P], BF16, kind="ExternalInput").ap()
    wv = nc.dram_tensor("wv", [P, DO, P], BF16, kind="ExternalInput").ap()
    wot = nc.dram_tensor("wot", [P, D], BF16, kind="ExternalInput").ap()
    bqk = nc.dram_tensor("bqk", [P, 2], F32, kind="ExternalInput").ap()
    wqkS = nc.dram_tensor("wqkS", [P, DO, 2, P], BF16, kind="ExternalInput").ap()
    bqkS = nc.dram_tensor("bqkS", [P, 2], F32, kind="ExternalInput").ap()
    out = nc.dram_tensor("out", [S, D], F16, kind="ExternalOutput").ap()
    _KVER = 20
    vw = 16 + 64 * _KVER + 4 * loop_n + reps + (0 if phase1 else 1) + (0 if phase2 else 2)
    ver = nc.dram_tensor("ver", [1, vw], F32, kind="ExternalOutput").ap()

    qt_r = qt.rearrange("(o p) s -> p o s", p=P)
    kt_r = kt.rearrange("(o p) s -> p o s", p=P)
    vt_r = vt.rearrange("(o p) s -> p o s", p=P)
    out_r = out.rearrange("(so p) m -> p so m", p=P)  # [128, 32, 1024]

    with tile.TileContext(nc) as tc, ExitStack() as ctx:
        const = ctx.enter_context(tc.tile_pool(name="const", bufs=1))
        pers = ctx.enter_context(tc.tile_pool(name="pers", bufs=1))
        pin = ctx.enter_context(tc.tile_pool(name="pin", bufs=3))
        pqd = ctx.enter_context(tc.tile_pool(name="pqd", bufs=2))
        pxs = ctx.enter_context(tc.tile_pool(name="pxs", bufs=2))
        pP = ctx.enter_context(tc.tile_pool(name="pP", bufs=5))
        pout = ctx.enter_context(tc.tile_pool(name="pout", bufs=3))
        prr = ctx.enter_context(tc.tile_pool(name="prr", bufs=2))
        psA = ctx.enter_context(tc.tile_pool(name="psA", bufs=1, space="PSUM"))
        psB = ctx.enter_context(tc.tile_pool(name="psB", bufs=1, space="PSUM"))
        psO = ctx.enter_context(tc.tile_pool(name="psO", bufs=1, space="PSUM"))
        psV = ctx.enter_context(tc.tile_pool(name="psV", bufs=1, space="PSUM"))

        wqk_sb = const.tile([P, DO, 2, P], BF16)
        nc.sync.dma_start(wqk_sb[:], wqk)
        wv_sb = const.tile([P, DO, P], BF16)
        nc.sync.dma_start(wv_sb[:], wv)
        wot_sb = const.tile([P, D], BF16)
        nc.sync.dma_start(wot_sb[:], wot)
        bqk_sb = const.tile([P, 2], F32)
        nc.sync.dma_start(bqk_sb[:], bqk)
        wqkS_sb = const.tile([P, DO, 2, P], BF16)
        nc.sync.dma_start(wqkS_sb[:], wqkS)
        bqkS_sb = const.tile([P, 2], F32)
        nc.sync.dma_start(bqkS_sb[:], bqkS)
        ones_sb = const.tile([1, DK], F32)
        nc.vector.memset(ones_sb[:], 1.0)
        ver_sb = const.tile([1, vw], F32)
        nc.vector.memset(ver_sb[:], float(vw))
        nc.sync.dma_start(ver, ver_sb[:])

        def body():
            qT = pers.tile([P, S], BF16, tag="qT", name="qT")
            kT = pers.tile([P, S], BF16, tag="kT", name="kT")
            kdup = pers.tile([P, S], BF16, tag="kdup", name="kdup")
            vx0 = pers.tile([P, NTB, DK + 1], F16, tag="vx0", name="vx0")
            vx1 = pers.tile([P, NTB, DK + 1], F16, tag="vx1", name="vx1")
            xT = pers.tile([P, S], BF16, tag="xT", name="xT")
            nc.vector.memset(vx0[:, :, DK], 1.0)
            nc.vector.memset(vx1[:, :, DK], 1.0)
            if not phase1:
                nc.vector.memset(qT[:], 0.01)
                nc.vector.memset(kT[:], 0.01)
                nc.vector.memset(kdup[:], 0.01)
                nc.vector.memset(vx0[:, :, 0:DK], 0.01)
                nc.vector.memset(vx1[:, :, 0:DK], 0.01)

            # ---------------- phase A: k + v projections ----------------
            for sb in range(NSB if phase1 else 0):
                s0 = sb * SB
                ks = pin.tile([P, DO, SB], BF16, tag="ks", name="ks")
                nc.sync.dma_start(ks[:], kt_r[:, :, s0 : s0 + SB])
                vs = pin.tile([P, DO, SB], BF16, tag="vs", name="vs")
                nc.sync.dma_start(vs[:], vt_r[:, :, s0 : s0 + SB])
                pk = psB.tile([P, 2 * SB], F32, tag="sB", name="pk")
                for o in range(DO):
                    nc.tensor.matmul(
                        pk[:, 0:SB], wqk_sb[:, o, 1, :], ks[:, o],
                        start=(o == 0), stop=(o == DO - 1),
                    )
                nc.vector.tensor_scalar(
                    kT[:, s0 : s0 + SB], pk[:, 0:SB],
                    bqk_sb[:, 1:2], None, mybir.AluOpType.add,
                )
                # head-swapped duplicate for QK row-tile alternation (no DMA)
                pk2 = psA.tile([P, 4 * SB], F32, tag="sA", name="pk2")
                for o in range(DO):
                    nc.tensor.matmul(
                        pk2[:, 0:SB], wqkS_sb[:, o, 1, :], ks[:, o],
                        start=(o == 0), stop=(o == DO - 1),
                    )
                nc.vector.tensor_scalar(
                    kdup[:, s0 : s0 + SB], pk2[:, 0:SB],
                    bqkS_sb[:, 1:2], None, mybir.AluOpType.add,
                )

                for tb in range(SB // TB):
                    if tb % 2 == 0:
                        pv = psV.tile([P, SB], F32, tag="pv", name="pvv")
                    else:
                        pv = psO.tile([P, SB], F32, tag="oT", name="pvo")
                    pvv = pv[:, 0:P]
                    tbg = sb * (SB // TB) + tb
                    for o in range(DO):
                        nc.tensor.matmul(
                            pvv, vs[:, o, tb * TB : (tb + 1) * TB], wv_sb[:, o],
                            start=(o == 0), stop=(o == DO - 1),
                        )
                    nc.vector.tensor_copy(vx0[:, tbg, 0:DK], pvv[:, 0:DK])
                    nc.vector.tensor_copy(vx1[:, tbg, 0:DK], pvv[:, DK:P])

            def do_outproj(sbp):
                for si in range(SB // P):
                    so = sbp * (SB // P) + si
                    for mb in range(2):
                        m0 = mb * SB
                        op = psV.tile([P, SB], F32, tag="pv", name="op")
                        nc.tensor.matmul(
                            op, xT[:, so * P : (so + 1) * P],
                            wot_sb[:, m0 : m0 + SB],
                            start=True, stop=True,
                        )
                        ob = pout.tile([P, SB], F16, tag="ob", name="ob")
                        nc.vector.tensor_copy(ob[:], op)
                        nc.sync.dma_start(out_r[:, so, m0 : m0 + SB], ob[:])

            # -------- phase B: q proj + attention + output projection --------
            for sb in range(NSB if phase2 else 0):
                s0 = sb * SB
                qs = pin.tile([P, DO, SB], BF16, tag="qs", name="qs")
                nc.sync.dma_start(qs[:], qt_r[:, :, s0 : s0 + SB])
                pq = psA.tile([P, 4 * SB], F32, tag="sA", name="pq")
                for o in range(DO):
                    nc.tensor.matmul(
                        pq[:, 0:SB], wqk_sb[:, o, 0, :], qs[:, o],
                        start=(o == 0), stop=(o == DO - 1),
                    )
                for o in range(DO):
                    nc.tensor.matmul(
                        pq[:, SB : 2 * SB], wqkS_sb[:, o, 0, :], qs[:, o],
                        start=(o == 0), stop=(o == DO - 1),
                    )
                nc.vector.tensor_scalar(
                    qT[:, s0 : s0 + SB], pq[:, 0:SB],
                    bqk_sb[:, 0:1], 0.125,
                    mybir.AluOpType.add, mybir.AluOpType.mult,
                )
                qd = pqd.tile([P, SB], BF16, tag="qd", name="qd")
                nc.vector.tensor_scalar(
                    qd[:], pq[:, SB : 2 * SB],
                    bqkS_sb[:, 0:1], 0.125,
                    mybir.AluOpType.add, mybir.AluOpType.mult,
                )

                for h in (1, 0):  # h1 first: its xT DMA-shift hides under h0
                    vx = vx0 if h == 0 else vx1
                    oT = psO.tile([DK + 1, SB], F32, tag="oT", name="oT")
                    starts = []
                    t = 0
                    for g in GROUPS:
                        starts.append(t)
                        t += g
                    pts = [None] * len(GROUPS)
                    NG = len(GROUPS)

                    def do_pv(gi):
                        g, gs = GROUPS[gi], starts[gi]
                        for i in range(g):
                            tb = gs + i
                            if PV_ALT:
                                for half in range(2):
                                    r0 = half * DK
                                    nc.tensor.matmul(
                                        oT, vx[r0 : r0 + DK, tb, :],
                                        pts[gi][r0 : r0 + DK, i * SB : (i + 1) * SB],
                                        start=(tb == 0 and half == 0),
                                        stop=(tb == NTB - 1 and half == 1),
                                    )
                            else:
                                nc.tensor.matmul(
                                    oT, vx[:, tb, :], pts[gi][:, i * SB : (i + 1) * SB],
                                    start=(tb == 0), stop=(tb == NTB - 1),
                                )

                    for gi, g in enumerate(GROUPS):
                        gs = starts[gi]
                        pool, width = (psB, 2 * SB) if g == 2 else (psA, 4 * SB)
                        sc = pool.tile([P, width], F32, tag=("sB" if g == 2 else "sA"), name="sc")
                        for i in range(g):
                            tb = gs + i
                            # row-tile position: h0 even tb -> 0, odd -> 64;
                            # h1 even tb -> 0 (kdup), odd -> 64 (kT)
                            pos = (tb % 2) * DK
                            native = pos == h * DK
                            k_src = kT if native else kdup
                            if native:
                                q_op = qT[pos : pos + DK, s0 : s0 + SB]
                            else:
                                q_op = qd[pos : pos + DK, :]
                            nc.tensor.matmul(
                                sc[:, i * SB : (i + 1) * SB],
                                k_src[pos : pos + DK, tb * TB : (tb + 1) * TB],
                                q_op,
                                start=True, stop=True,
                            )
                        pt = pP.tile([P, 4 * SB], F16, tag="P", name="pt")
                        nc.scalar.activation(
                            pt[:, : g * SB], sc[:, : g * SB],
                            mybir.ActivationFunctionType.Exp,
                        )
                        pts[gi] = pt
                        if gi >= PV_LAG:
                            do_pv(gi - PV_LAG)
                    for gi in range(NG - PV_LAG, NG):
                        do_pv(gi)

                    # r = 1/l ; broadcast over 64 partitions ; xT = oT * r
                    r_t = prr.tile([1, SB], F32, tag="r", name="r_t")
                    r_s = prr.tile([1, SB], F32, tag="rs", name="r_s")
                    l_t = prr.tile([1, SB], F32, tag="lt", name="l_t")
                    nc.vector.tensor_copy(l_t[:], oT[DK : DK + 1, :])
                    nc.vector.reciprocal_approx_accurate(r_t[:], l_t[:], r_s[:])
                    bc = psV.tile([P, SB], F32, tag="pv", name="bc")
                    nc.tensor.matmul(bc[0:DK, :], ones_sb[:], r_t[:], start=True, stop=True)
                    bc_sb = prr.tile([DK, SB], F32, tag="bcs", name="bc_sb")
                    nc.vector.tensor_copy(bc_sb[:], bc[0:DK, :])
                    if h == 0:
                        nc.vector.tensor_tensor(
                            xT[0:DK, s0 : s0 + SB], oT[0:DK, :], bc_sb[:],
                            mybir.AluOpType.mult,
                        )
                    else:
                        xst = pxs.tile([DK, SB], BF16, tag="xst", name="xst")
                        nc.vector.tensor_tensor(
                            xst[:], oT[0:DK, :], bc_sb[:], mybir.AluOpType.mult
                        )
                        nc.sync.dma_start(xT[DK:P, s0 : s0 + SB], xst[:])

                # output projection delayed one s-block: xT(sb-1) has had a
                # full attention pair of slack (covers the h1 xst DMA shift)
                if sb > 0:
                    do_outproj(sb - 1)
            if phase2:
                do_outproj(NSB - 1)

        if loop_n > 0:
            with tc.For_i(0, loop_n, 1):
                body()
        else:
            for _ in range(reps):
                body()

    nc.finalize()
    return nc


def _pack_core_inputs(c, QT, KT, VT, Wq, bq, Wk, bk, Wv, Wo):
    """Per-core input dict (core c owns heads 2c, 2c+1)."""
    h0 = HL * c
    # wqk[p, o, qk, j]: j = (head, dk) packed
    wqk = np.zeros((P, DO, 2, P), dtype=BF16_NP)
    for hh in range(HL):
        wqk[:, :, 0, hh * DK : (hh + 1) * DK] = (
            Wq[h0 + hh].reshape(DO, P, DK).transpose(1, 0, 2).astype(BF16_NP)
        )
        wqk[:, :, 1, hh * DK : (hh + 1) * DK] = (
            Wk[h0 + hh].reshape(DO, P, DK).transpose(1, 0, 2).astype(BF16_NP)
        )
    wv = (
        Wv[h0 : h0 + HL].reshape(HL, DO, P, DK).transpose(2, 1, 0, 3).reshape(P, DO, P)
    ).astype(BF16_NP)
    # wot[j, m]: rows = (head, dk) packed
    wot = np.ascontiguousarray(Wo[:, h0 * DK : (h0 + HL) * DK].T).astype(BF16_NP)
    bqk = np.zeros((P, 2), dtype=np.float32)
    for hh in range(HL):
        bqk[hh * DK : (hh + 1) * DK, 0] = bq[h0 + hh]
        bqk[hh * DK : (hh + 1) * DK, 1] = bk[h0 + hh]
    # head-swapped copies (duplicate rows for PE row-tile alternation)
    wqkS = np.concatenate([wqk[:, :, :, DK:], wqk[:, :, :, :DK]], axis=3)
    bqkS = np.concatenate([bqk[DK:], bqk[:DK]], axis=0)
    return {
        "qt": QT, "kt": KT, "vt": VT,
        "wqk": np.ascontiguousarray(wqk),
        "wv": np.ascontiguousarray(wv),
        "wot": wot,
        "bqk": np.ascontiguousarray(bqk),
        "wqkS": np.ascontiguousarray(wqkS),
        "bqkS": np.ascontiguousarray(bqkS),
    }


def make_in_maps(Q, K, V, Wq, bq, Wk, bk, Wv, bv, Wo, bo):
    QT = np.ascontiguousarray(Q.T).astype(BF16_NP)
    KT = np.ascontiguousarray(K.T).astype(BF16_NP)
    VT = np.ascontiguousarray(V.T).astype(BF16_NP)
    return [
        _pack_core_inputs(c, QT, KT, VT, Wq, bq, Wk, bk, Wv, Wo) for c in range(NC)
    ]


def host_combine(partials, Wq, bv, Wo, bo):
    total = np.zeros((S, D), np.float32)
    for p in partials:
        total += p.astype(np.float32)
    # v-bias passes through softmax exactly as +bv on the concat features
    total += bv.reshape(-1).astype(np.float32) @ Wo.T.astype(np.float32) + bo
    return total


_NC_CACHE = {}


def _get_nc(reps=1):
    if reps not in _NC_CACHE:
        _NC_CACHE[reps] = build_nc(reps)
    return _NC_CACHE[reps]


def kernel(Q, K, V, Wq, bq, Wk, bk, Wv, bv, Wo, bo):
    args = [np.asarray(x) for x in (Q, K, V, Wq, bq, Wk, bk, Wv, bv, Wo, bo)]
    Q, K, V, Wq, bq, Wk, bk, Wv, bv, Wo, bo = args
    nc = _get_nc()
    in_maps = make_in_maps(Q, K, V, Wq, bq, Wk, bk, Wv, bv, Wo, bo)
    res = run_bass_kernel_spmd(nc, in_maps, core_ids=list(range(NC)))
    partials = [res.results[c]["out"] for c in range(NC)]
    return host_combine(partials, Wq, bv, Wo, bo)
